# revision 36
# baseline (speedup 1.0000x reference)
"""Multi-head scaled-dot-product attention on 8 Trainium2 NeuronCores.

Problem: x[4,2048,128], Wq/Wk/Wv[10,128,128] (torch Linear layout [e_out,d_in]),
Wo[128,1280], bo[128]  ->  out[4,2048,128]

Sharding: 8 cores = 4 batches x 2 head-groups (5 heads each). Each core
computes its batch's attention for its 5 heads plus the partial output
projection; the host sums the two half-head partials per batch, transposes,
and adds the bias.

Math folding (host side, per head h):
  A_h  = Wq_h^T @ Wk_h          [D,D]   (one projection replaces Q and K:
         S = X Wq^T Wk X^T = G X^T with G = X A)
  W2_h = Wv_h^T @ Wo_h^T        [D,D]   (V-projection folded into out-proj:
         out_h = (P X) W2_h)

Per-core layout (all host-side pre-transposed; zero on-chip transposes):
  xT   [d,n]    = x[b].T            lhsT chunks for scores, rhs for proj
  xn   [p,c,d]  : xn[p,c,:] = x[c*128+p,:]   lhsT chunks for P@X
  gt   [d2,n]   = A^T X^T per head  (proj matmul: lhsT=A_h, rhs=xT block)
  ST   [k-chunk, q-blk] = xT_chunk.T @ gt_blk   (scores, keys on partitions)
  PT   = exp(ST / sqrt(D))          (ACT; scores ~N(0,1), exp safe in fp32)
  OT   [d, q-blk] += xn_chunk.T @ PT_chunk      (accumulated over 16 chunks)
  den  via DVE pair-accumulator + 2 ones-matmuls; reciprocal on DVE
  outT [dout, q-blk] += W2_h^T-style matmul over 5 heads, then DMA out

Emission is software-pipelined with a 2-tile score lookahead so the ACT
engine (the critical engine: 21M exp elements/core) never waits on PE.
"""

from collections import deque

import numpy as np

import concourse.tile as tile
from concourse import bacc, mybir
from concourse.bass import ds, ts
from concourse.bass_utils import run_bass_kernel_spmd

B, N, D, H = 4, 2048, 128, 10
HL = H // 2  # heads per core
NCHUNK = N // 128  # 16 key chunks
NBLK = N // 512  # 4 query blocks
NPAIR = NCHUNK // 2  # 8 chunk-pairs per (nb, h)
INV_SCALE = float(1.0 / (128.0**0.5 + 1e-8))
f32 = mybir.dt.float32
fp16 = mybir.dt.float16

PROFILE = False
LAST_RESULTS = None

_built = None


def _emit(tc, xT, xn, m_in, w2, ones_dram, outT):
    nc = tc.nc
    Exp = mybir.ActivationFunctionType.Exp

    from contextlib import ExitStack

    ctx = ExitStack()
    consts = ctx.enter_context(tc.tile_pool(name="consts", bufs=1))
    proj = ctx.enter_context(tc.tile_pool(name="proj", bufs=1))
    ps = ctx.enter_context(tc.tile_pool(name="ps", bufs=2, space="PSUM"))
    otps = ctx.enter_context(tc.tile_pool(name="otps", bufs=2, space="PSUM"))
    dnps = ctx.enter_context(tc.tile_pool(name="dnps", bufs=1, space="PSUM"))
    outps = ctx.enter_context(tc.tile_pool(name="outps", bufs=1, space="PSUM"))
    ptp = ctx.enter_context(tc.tile_pool(name="ptp", bufs=6))
    work = ctx.enter_context(tc.tile_pool(name="work", bufs=2))

    # DMA-written tiles get whole-tile (not subtile) dependency tracking, so
    # anything loaded by several DMAs is split into separate tiles: consumers
    # must not wait for sibling loads they don't read.
    ones_mat = consts.tile([128, 128], fp16)
    xT_a = consts.tile([D, 512], fp16)  # query-block 0 / key chunks 0-3
    xT_b = consts.tile([D, N - 512], fp16)  # the rest of xT
    xn_cs = [
        consts.tile([D, 128], fp16, name=f"xn{c}") for c in range(NCHUNK)
    ]  # xn_cs[c][p, d] = x[c*128+p, d]
    m0_sb = consts.tile([D, D], fp16)
    mr_sb = consts.tile([D, (HL - 1) * D], fp16)
    w2_sb = consts.tile([D, HL * D], fp16)

    def m_head(h):
        return m0_sb[:] if h == 0 else mr_sb[:, ts(h - 1, D)]

    def xT_chunk(cc):
        # key chunk cc = xT columns [cc*128, cc*128+128)
        if cc < 4:
            return xT_a[:, ts(cc, 128)]
        return xT_b[:, ts(cc - 4, 128)]

    def xT_block(j):
        # query block j = xT columns [j*512, j*512+512)
        if j == 0:
            return xT_a[:]
        return xT_b[:, ts(j - 1, 512)]

    # --- input DMAs; descriptor generation (~600ns each) serializes per
    # queue, so order by first-use. The scalar queue carries NO input DMAs:
    # its sequencer must be free for the first PSUM evacuation + EXPs.
    # The two DMAs that gate the first projection go first on sync; the
    # bulk loads are emitted after the first score tiles. (Keep input DMAs
    # off the scalar queue entirely — queue management there inflates every
    # subsequent ACT instruction.)
    nc.sync.dma_start(xT_a[:], xT[:, ts(0, 512)])
    nc.sync.dma_start(m0_sb[:], m_in[0])

    def emit_late_dmas():
        # NOP splits the sync queue's DMA batch so {xT_a, m0}'s completion
        # semaphore fires as soon as they land, not after the bulk loads
        nc.sync.nop()
        nc.sync.dma_start(xT_b[:], xT[:, ds(512, 1536)])
        nc.sync.dma_start(
            mr_sb[:].rearrange("p (h e) -> p h e", h=HL - 1),
            m_in[1:].rearrange("h d e -> d h e"),
        )
        # xn chunks: clean [128 part, 256B] natural-layout blocks; chunk c
        # is first used at PX step c, well after its descriptor lands
        xn_src = xn.rearrange("(c p) d -> p c d", p=128)
        for c in range(NCHUNK):
            nc.gpsimd.dma_start(xn_cs[c][:], xn_src[:, c])
        nc.gpsimd.dma_start(ones_mat[:], ones_dram)
        nc.gpsimd.dma_start(
            w2_sb[:].rearrange("p (h e) -> p h e", h=HL),
            w2.rearrange("h d e -> d h e"),
        )

    gt = proj.tile([D, HL * N], fp16)

    # --- projection jobs: gt[:, h*N + j*512] = A_h^T-contracted block ---
    # head 0 runs upfront (gates the very first scores); heads 1..4 are
    # interleaved in PAIRS on even iterations of head-0's chunk stream:
    # the ps slot ring advances per tile allocation, so only an even
    # number of interlopers per iteration keeps consecutive score tiles
    # on alternating slots (odd counts serialize the score pipeline).
    def proj_job(h, j, scalar_evac=False):
        # interleaved jobs ride the dnps bank's tag ring (idle until each
        # head's end) so they never perturb the score-slot alternation
        if scalar_evac:
            p = ps.tile([128, 1024], f32, tag="st", name="projp")[:, ts(0, 512)]
        else:
            p = dnps.tile([128, 512], f32, tag="dn_ps", name="projp")
        nc.tensor.matmul(
            p,
            m_head(h),
            xT_block(j),
            start=True,
            stop=True,
        )
        dst = gt[:, ds(h * N + j * 512, 512)]
        if scalar_evac:
            nc.scalar.copy(dst, p)
        else:
            nc.vector.tensor_copy(dst, p)

    # Only gt[:, h*N + 0:512] (query block 0) is needed for the first 40
    # iterations — emit just P(0,0) upfront; everything else interleaves.
    proj_job(0, 0, scalar_evac=True)
    # PE warmup in the dead window between P(0,0) and the first scores
    # (which wait on the bulk-DMA batch): HAM un-throttles ~3.4us after
    # sustained activity starts, so burn the wait in the idle outp bank
    # and the early score stream runs at 2.4GHz instead of 1.2GHz.
    warm = outps.tile([128, 512], f32, tag="outp", name="warm")
    for _ in range(20):
        nc.tensor.matmul(
            warm[:, ts(0, 128)], m0_sb[:], m0_sb[:], start=True, stop=True
        )
    proj_jobs = deque(
        [(h, 0) for h in range(1, HL)]
        + [(h, j) for j in range(1, NBLK) for h in range(HL)]
    )

    # --- flat score-tile schedule with lookahead-2 production ---
    tiles = [(nb, h, cp) for nb in range(NBLK) for h in range(HL) for cp in range(NPAIR)]
    NT = len(tiles)
    st_tiles = {}

    def produce(i):
        if i >= NT:
            return
        nb, h, cp = tiles[i]
        stp = ps.tile([128, 1024], f32, tag="st", name=f"st{i}")
        for j in range(2):
            nc.tensor.matmul(
                stp[:, ts(j, 512)],
                xT_chunk(2 * cp + j),
                gt[:, ds(h * N + nb * 512, 512)],
                start=True,
                stop=True,
            )
        st_tiles[i] = stp

    # per-(nb,h) state
    pend = None  # epilogue of the previous head

    def emit_finish(st):
        nb, h = st["nb"], st["h"]
        last_of_all = nb == NBLK - 1 and h == HL - 1
        if last_of_all:
            # overlap the exposed final epilogue: W2 on UNnormalized OT in
            # parallel with the dn/recip chain, then one fused combine
            otc = work.tile([128, 512], fp16, tag="otn")
            nc.vector.tensor_copy(otc[:], st["ot_ps"][:])
            w2u = ps.tile([128, 1024], f32, tag="st", name="w2u")
            nc.tensor.matmul(
                w2u[:, ts(0, 512)], w2_sb[:, ts(h, D)], otc[:], start=True, stop=True
            )
            t1 = work.tile([128, 512], f32, tag="t1")
            nc.vector.tensor_mul(t1[:], w2u[:, ts(0, 512)], st["bc"][:])
            osb = work.tile([128, 512], f32, tag="osb")
            nc.vector.tensor_add(osb[:], st["outp"][:], t1[:])
            nc.sync.dma_start(outT[:, ts(nb, 512)], osb[:])
            return
        otn = work.tile([128, 512], fp16, tag="otn")
        nc.vector.tensor_mul(otn[:], st["ot_ps"][:], st["bc"][:])
        stop_h = HL - 2 if nb == NBLK - 1 else HL - 1
        nc.tensor.matmul(
            st["outp"][:],
            w2_sb[:, ts(h, D)],
            otn[:],
            start=(h == 0),
            stop=(h == stop_h),
        )
        if h == HL - 1:
            osb = work.tile([128, 512], f32, tag="osb")
            nc.vector.tensor_copy(osb[:], st["outp"][:])
            nc.sync.dma_start(outT[:, ts(nb, 512)], osb[:])

    PRE = 2
    for i in range(PRE):
        produce(i)
    emit_late_dmas()

    ot_ps = acc = outp = None
    for i, (nb, h, cp) in enumerate(tiles):
        if cp == 0:
            ot_ps = otps.tile([128, 512], f32, tag="ot_ps")
            if h == 0:
                outp = outps.tile([128, 512], f32, tag="outp")
        stp = st_tiles.pop(i)
        p = ptp.tile([128, 1024], fp16, tag="pt")
        nc.scalar.activation(p[:], stp[:], Exp, scale=INV_SCALE)
        produce(i + PRE)
        # interleave remaining projections one per iteration (from i=2, once
        # the PE is HAM-warm); they ride the dnps ring so score-slot parity
        # is untouched
        if proj_jobs and i >= 2:
            proj_job(*proj_jobs.popleft())
        # P @ X accumulation for this pair
        for j in range(2):
            cc = 2 * cp + j
            nc.tensor.matmul(
                ot_ps[:],
                xn_cs[cc][:],
                p[:, ts(j, 512)],
                start=(cc == 0),
                stop=(cc == NCHUNK - 1),
            )
        # denominator pair-accumulator on DVE; for the very last head the
        # final pair goes straight into extra dn matmuls (PE is idle in the
        # tail) so the dn chain starts one DVE-ADD earlier
        last_of_all = nb == NBLK - 1 and h == HL - 1
        if cp == 0:
            acc = work.tile([128, 1024], fp16, tag="dacc")
            nc.vector.tensor_copy(acc[:], p[:])
        elif not (last_of_all and cp == NPAIR - 1):
            nc.vector.tensor_add(acc[:], acc[:], p[:])
        # previous head's epilogue, mid-stream where PE has slack
        if pend is not None and cp == 4:
            emit_finish(pend)
            pend = None
        if cp == NPAIR - 1:
            dn_ps = dnps.tile([128, 512], f32, tag="dn_ps")
            dn_srcs = [acc[:, ts(0, 512)], acc[:, ts(1, 512)]]
            if last_of_all:
                dn_srcs += [p[:, ts(0, 512)], p[:, ts(1, 512)]]
            for j, src in enumerate(dn_srcs):
                nc.tensor.matmul(
                    dn_ps[:],
                    ones_mat[:],
                    src,
                    start=(j == 0),
                    stop=(j == len(dn_srcs) - 1),
                )
            bc = work.tile([128, 512], f32, tag="bc")
            nc.vector.reciprocal_approx_fast(out=bc[:], in_=dn_ps[:])
            pend = {"ot_ps": ot_ps, "bc": bc, "outp": outp, "h": h, "nb": nb}
    emit_finish(pend)
    ctx.close()


def _build():
    nc = bacc.Bacc("TRN2", target_bir_lowering=False, debug=False)
    xT = nc.dram_tensor("xT", [D, N], fp16, kind="ExternalInput").ap()
    xn = nc.dram_tensor("xn", [N, D], fp16, kind="ExternalInput").ap()
    m_in = nc.dram_tensor("m_in", [HL, D, D], fp16, kind="ExternalInput").ap()
    w2 = nc.dram_tensor("w2", [HL, D, D], fp16, kind="ExternalInput").ap()
    ones_dram = nc.dram_tensor("ones", [D, D], fp16, kind="ExternalInput").ap()
    outT = nc.dram_tensor("outT", [D, N], f32, kind="ExternalOutput").ap()
    with tile.TileContext(nc) as tc:
        with nc.allow_low_precision(reason="fp16 matmul operands"):
            _emit(tc, xT, xn, m_in, w2, ones_dram, outT)
    nc.compile()
    return nc


def kernel(x, Wq, Wk, Wv, Wo, bo):
    global _built, LAST_RESULTS
    x = np.asarray(x, dtype=np.float32)
    Wq = np.asarray(Wq, dtype=np.float32)
    Wk = np.asarray(Wk, dtype=np.float32)
    Wv = np.asarray(Wv, dtype=np.float32)
    Wo = np.asarray(Wo, dtype=np.float32)
    bo = np.asarray(bo, dtype=np.float32)

    if _built is None:
        _built = _build()
    nc = _built

    # A_h = Wq_h^T @ Wk_h ; W2_h = Wv_h^T @ Wo_h^T
    A = np.einsum("hed,hef->hdf", Wq, Wk).astype(np.float16)
    W2 = np.einsum(
        "hde,heo->hdo", Wv.transpose(0, 2, 1), Wo.T.reshape(H, D, D)
    ).astype(np.float16)
    A = np.ascontiguousarray(A)
    W2 = np.ascontiguousarray(W2)

    in_maps = []
    for c in range(8):
        b, g = divmod(c, 2)
        hsl = slice(g * HL, g * HL + HL)
        in_maps.append(
            {
                "xT": np.ascontiguousarray(x[b].T.astype(np.float16)),
                "xn": np.ascontiguousarray(x[b].astype(np.float16)),
                "m_in": A[hsl],
                "w2": W2[hsl],
                "ones": np.ones((D, D), dtype=np.float16),
            }
        )

    res = run_bass_kernel_spmd(
        nc, in_maps, core_ids=list(range(8)), trace=PROFILE
    )
    LAST_RESULTS = res

    out = np.empty((B, N, D), dtype=np.float32)
    for b in range(B):
        oT = res.results[2 * b]["outT"] + res.results[2 * b + 1]["outT"]
        out[b] = oT.T
    out += bo
    return out
